# revision 28
# baseline (speedup 1.0000x reference)
"""Euler-characteristic-curve kernel for Trainium2 (Bass/Tile), v2.

Per (batch, channel) group the reference computes
    cover(t_k) = #{n : birth_n < t_k <= death_n},  t_k = k/255 (f32)
and the output is cover_pd0 - cover_pd1.

Identity: [b < t][d >= t] = [b < t] - [max(b,d) < t], so everything
reduces to cumulative counts C(t_k) = #{v : v < t_k} of value streams.

Exact bin index per value: q = round(v*255) - [v < t_c] (int16), with
t_c = f32(c) * f32(1/255) matching the reference grid bit-exactly.

Counting scheme: with q = 16*qh + ql,
    C(16K+L) = Cc(K) + sum_p [qh_p == K][ql_p < L],   Cc = prefix(hist(qh))
Per 128-point pass the PE contracts
    A[p, .] = one-hot(qh)      (is_equal vs immediate, DVE 4x mode)
    B[p, .] = thermometer(ql)  (is_lt vs immediate, DVE 4x; col 0 = ones)
so PSUM accumulates, per stream, M[K,L] = joint prefix counts and
M[K,0] = hist(qh); the thermometer makes per-row scans unnecessary.
Rows/cols are interleaved (DG*bin + slot) so each pass's operand AP is a
single uniform-stride free dim.  Birth values accumulate into M0, max
values into M1; og = M0[d0]-M1[d0]-M0[d1]+M1[d1] (all four signs) falls
out of [+sel|-sel] selection matmuls at postproc, then one tiny
triangular matmul gives Cc and a broadcast-add finishes C.

Postprocessing of set s is emitted after compute of set s+1 so the
in-order ACT/DVE/PE streams never stall waiting on PSUM stops.

Sharding: data-parallel over batch, 4 batches per core x 8 cores.
"""

import os
import sys

for _p in ("/opt/trn_rl_repo", os.path.expanduser("~/.axon_site/_ro/trn_rl_repo")):
    if os.path.isdir(_p) and _p not in sys.path:
        sys.path.insert(0, _p)

import numpy as np

import concourse.bass as bass
import concourse.bacc as bacc
import concourse.mybir as mybir
from concourse.tile import TileContext
from concourse.bass_utils import run_bass_kernel_spmd

NCORES = 8
B, C, N = 32, 3, 8192
TT = 256
NG = (B // NCORES) * C        # 12 groups (b,c pairs) per diagram per core
R = float(np.float32(1.0) / np.float32(255.0))
SIZES = [4, 4, 4]             # groups per set (sum = NG, each <= 4)

F32 = mybir.dt.float32
BF16 = mybir.dt.bfloat16
I16 = mybir.dt.int16
OP = mybir.AluOpType
AF = mybir.ActivationFunctionType
P23 = 8388608.0               # 2^23


def build_nc():
    nc = bacc.Bacc("TRN2", target_bir_lowering=False, debug=False)
    pds = [
        nc.dram_tensor(f"pd{d}", [NG, N, 2], F32, kind="ExternalInput")
        for d in range(2)
    ]
    tri_d = nc.dram_tensor("tri", [16, 16], F32, kind="ExternalInput")
    sel_d = nc.dram_tensor("sel", [128, 256], F32, kind="ExternalInput")
    out_d = nc.dram_tensor("out", [NG, TT], F32, kind="ExternalOutput")

    with TileContext(nc) as tc:
        with (
            tc.tile_pool(name="consts", bufs=1) as cpool,
            tc.tile_pool(name="src", bufs=2) as spool,
            tc.tile_pool(name="prep", bufs=2) as tpool,
            tc.tile_pool(name="oh", bufs=2) as ohpool,
            tc.tile_pool(name="mm", bufs=2, space="PSUM") as ppool,
            tc.tile_pool(name="pcc", bufs=2, space="PSUM") as ccpool,
            tc.tile_pool(name="post", bufs=2) as qpool,
        ):
            tri = cpool.tile([16, 16], F32)
            sel = cpool.tile([128, 256], F32)

            goffs = np.cumsum([0] + SIZES[:-1]).tolist()

            def prep(sd):
                """DMA + bin-index computation for set sd."""
                G = SIZES[sd]
                goff = goffs[sd]
                W = 256 * G
                DG = 2 * G

                src = spool.tile([128, 2, G, 128], F32, tag="src", name="src")
                for d in range(2):
                    nc.sync.dma_start(
                        src[:, d, :, :],
                        pds[d]
                        .ap()[goff : goff + G, :, :]
                        .rearrange("g (p x) two -> p g (x two)", p=128),
                    )
                if sd == 0:
                    nc.sync.dma_start(tri[:, :], tri_d.ap())
                    nc.sync.dma_start(sel[:, :], sel_d.ap())

                flat = src[:, :, :, :].rearrange("p d g x -> p (d g x)")
                pairs = src[:, :, :, :].rearrange(
                    "p d g (i two) -> p (d g i) two", two=2
                )
                # deaths <- max(birth, death) in place
                nc.vector.tensor_tensor(
                    pairs[:, :, 1:2], pairs[:, :, 0:1], pairs[:, :, 1:2],
                    OP.max,
                )

                # exact bin index: c = round(v*255) via +2^23 trick; true
                # index is c - [v < t_c] with t_c = fl(c * R)
                tmb = tpool.tile([128, W], F32, tag="tmb", name="tmb")
                cf = tpool.tile([128, W], F32, tag="cf", name="cf")
                tlo = tpool.tile([128, W], F32, tag="tlo", name="tlo")
                ltf = tpool.tile([128, W], F32, tag="ltf", name="ltf")
                q = tpool.tile([128, W], I16, tag="q", name="q")
                nc.scalar.activation(
                    tmb[:, :], flat[:, :], AF.Copy, bias=P23, scale=255.0
                )
                nc.scalar.activation(cf[:, :], tmb[:, :], AF.Copy, bias=-P23)
                nc.scalar.mul(tlo[:, :], cf[:, :], float(R))
                nc.vector.tensor_tensor(
                    ltf[:, :], flat[:, :], tlo[:, :], OP.is_lt
                )
                nc.vector.tensor_tensor(q[:, :], cf[:, :], ltf[:, :], OP.subtract)
                qh = tpool.tile([128, W], I16, tag="qh", name="qh")
                ql = tpool.tile([128, W], I16, tag="ql", name="ql")
                nc.vector.tensor_scalar(
                    qh[:, :], q[:, :], 4, None, OP.logical_shift_right
                )
                nc.vector.tensor_scalar(
                    ql[:, :], q[:, :], 15, None, OP.bitwise_and
                )
                return (sd, G, goff, W, DG, q, qh, ql)

            def bins(st):
                """One-hot/thermometer bin tensors (DVE 4x ops)."""
                sd, G, goff, W, DG, q, qh, ql = st
                # bin tensors, [p, bin, d, g, i, v]; all DVE 4x ops
                A = ohpool.tile([128, 16, 2, G, 64, 2], BF16, tag="A", name="A")
                Bt = ohpool.tile([128, 16, 2, G, 64, 2], BF16, tag="B", name="B")
                Af = A[:, :, :, :, :, :].rearrange("p h d g i v -> p (h d g i v)")
                Bf = Bt[:, :, :, :, :, :].rearrange("p h d g i v -> p (h d g i v)")
                # ones col (L=0): always-true compare runs in DVE 4x mode
                # (memset would be 4x slower and buffer reuse trips the
                # race detector)
                nc.vector.tensor_scalar(
                    Bf[:, 0:W], q[:, :], 30000, None, OP.is_lt
                )
                for h in range(16):
                    nc.vector.tensor_scalar(
                        Af[:, W * h : W * (h + 1)], qh[:, :],
                        h, None, OP.is_equal,
                    )
                for L in range(1, 16):
                    nc.vector.tensor_scalar(
                        Bf[:, W * L : W * (L + 1)], ql[:, :],
                        L, None, OP.is_lt,
                    )
                return (sd, G, goff, DG, A, Bt)

            def mains(bt):
                """Main accumulation matmuls for set sd."""
                sd, G, goff, DG, A, Bt = bt
                # PE: per (v, i) pass contract 128 points; interleaved
                # rows DG*h + slot, cols DG*L + slot (slot = G*d + g)
                M = [ppool.tile([DG * 16, DG * 16], F32, tag=f"M{v}",
                                name=f"M{v}")
                     for v in range(2)]

                def mk_ap(t, i, v):
                    ap = t[:, :, :, :, i, v]
                    # single free dim: addr = 128*(DG*bin + G*d + g)
                    return bass.AP(
                        ap.tensor, ap.offset,
                        [ap.ap[0], [ap.ap[3][0], DG * 16]],
                    )

                for v in range(2):
                    for i in range(64):
                        nc.tensor.matmul(
                            M[v][:, :],
                            mk_ap(A, i, v),
                            mk_ap(Bt, i, v),
                            start=(i == 0),
                            stop=(i == 63),
                        )
                return (sd, G, goff, DG, M)

            def netcp(state):
                """PSUM -> SBUF copies (ACT); one PSUM read per op only."""
                sd, G, goff, DG, M = state
                net0 = qpool.tile([DG * 16, DG * 16], F32, tag="net0",
                                  name="net0")
                net1 = qpool.tile([DG * 16, DG * 16], F32, tag="net1",
                                  name="net1")
                nc.scalar.copy(net0[:, :], M[0][:, :])
                nc.scalar.copy(net1[:, :], M[1][:, :])
                return (sd, G, goff, DG, net0, net1)

            def extract(ns):
                sd, G, goff, DG, net0, net1 = ns
                # og_g = +M0[d0] - M1[d0] - M0[d1] + M1[d1] via selection
                # matmuls; sel[p, 16s+K] = [p == DG*K + s] (negated at +128)
                ogp = ccpool.tile([16, G, 16], F32, tag="ogp", name="ogp")

                def blkcols(net, s):
                    ap = net[:, :]
                    return bass.AP(
                        ap.tensor, ap.offset + s * ap.ap[1][0],
                        [ap.ap[0], [DG * ap.ap[1][0], 16]],
                    )

                for g in range(G):
                    s0, s1 = g, G + g
                    for k, (nt, s, neg) in enumerate(
                        ((net0, s0, 0), (net1, s0, 1),
                         (net0, s1, 1), (net1, s1, 0))
                    ):
                        c0 = 128 * neg + 16 * s
                        nc.tensor.matmul(
                            ogp[:, g, :],
                            sel[:, c0 : c0 + 16],
                            blkcols(nt, s),
                            start=(k == 0), stop=(k == 3),
                        )
                og = qpool.tile([16, G, 16], F32, tag="og", name="og")
                nc.scalar.copy(og[:, :, :], ogp[:, :, :])
                # coarse prefix: Cc[K, g] = sum_{K'<K} hist[K', g]
                ccp = ccpool.tile([16, G], F32, tag="ccp", name="ccp")
                nc.tensor.matmul(
                    ccp[:, :], tri[:, :], og[:, :, 0], start=True, stop=True
                )
                fin = qpool.tile([16, G, 16], F32, tag="fin", name="fin")
                nc.scalar.copy(fin[:, :, 0], ccp[:, :])
                return (sd, G, goff, og, fin)

            def fin_phase(ex):
                sd, G, goff, og, fin = ex
                for g in range(G):
                    nc.vector.tensor_scalar(
                        fin[:, g, 1:16], og[:, g, 1:16],
                        fin[:, g, 0:1], None, OP.add,
                    )
                nc.sync.dma_start(
                    out_d.ap()[goff : goff + G, :].rearrange(
                        "g (K L) -> K g L", K=16
                    ),
                    fin[:, :, :],
                )

            # per-engine interleaving: each engine's in-order queue sees
            # work in data-readiness order (prep(s+1) before netcp(s) on
            # ACT; extract(s) between mains(s) and mains(s+1) on PE; fin(s)
            # after bins(s+1) on DVE)
            NS = len(SIZES)
            mstate = {}
            exstate = {}
            for sd in range(NS):
                st = prep(sd)
                if sd >= 1:
                    exstate[sd - 1] = extract(netcp(mstate[sd - 1]))
                bt = bins(st)
                if sd >= 1:
                    fin_phase(exstate[sd - 1])
                mstate[sd] = mains(bt)
            exstate[NS - 1] = extract(netcp(mstate[NS - 1]))
            fin_phase(exstate[NS - 1])
    nc.compile()
    return nc


_NC = None


def _get_nc():
    global _NC
    if _NC is None:
        _NC = build_nc()
    return _NC


def make_in_maps(pd0, pd1):
    pd0 = np.ascontiguousarray(np.asarray(pd0, dtype=np.float32))
    pd1 = np.ascontiguousarray(np.asarray(pd1, dtype=np.float32))
    tri = (np.arange(16)[:, None] < np.arange(16)[None, :]).astype(np.float32)
    # sel[8K + s, 16s + K] = +1 (cols 0..127), -1 at cols 128..255
    sel = np.zeros((128, 256), dtype=np.float32)
    for s in range(8):
        for K in range(16):
            sel[8 * K + s, 16 * s + K] = 1.0
            sel[8 * K + s, 128 + 16 * s + K] = -1.0
    bs = B // NCORES
    in_maps = []
    for c in range(NCORES):
        in_maps.append(
            {
                "pd0": np.ascontiguousarray(
                    pd0[bs * c : bs * (c + 1)].reshape(NG, N, 2)
                ),
                "pd1": np.ascontiguousarray(
                    pd1[bs * c : bs * (c + 1)].reshape(NG, N, 2)
                ),
                "tri": tri,
                "sel": sel,
            }
        )
    return in_maps


def kernel(pd0, pd1, trace=False):
    nc = _get_nc()
    in_maps = make_in_maps(pd0, pd1)
    res = run_bass_kernel_spmd(nc, in_maps, list(range(NCORES)), trace=trace)
    bs = B // NCORES
    out = np.concatenate(
        [res.results[c]["out"].reshape(bs, C, TT) for c in range(NCORES)], axis=0
    )
    if trace:
        return out.astype(np.float32), res
    return out.astype(np.float32)


# revision 34
# speedup vs baseline: 1.0876x; 1.0876x over previous
"""Euler-characteristic-curve kernel for Trainium2 (Bass/Tile), v2.

Per (batch, channel) group the reference computes
    cover(t_k) = #{n : birth_n < t_k <= death_n},  t_k = k/255 (f32)
and the output is cover_pd0 - cover_pd1.

Identity: [b < t][d >= t] = [b < t] - [max(b,d) < t], so everything
reduces to cumulative counts C(t_k) = #{v : v < t_k} of value streams.

Exact bin index per value: q = round(v*255) - [v < t_c] (int16), with
t_c = f32(c) * f32(1/255) matching the reference grid bit-exactly.

Counting scheme: with q = 16*qh + ql,
    C(16K+L) = Cc(K) + sum_p [qh_p == K][ql_p < L],   Cc = prefix(hist(qh))
Per 128-point pass the PE contracts
    A[p, .] = one-hot(qh)      (is_equal vs immediate, DVE 4x mode)
    B[p, .] = thermometer(ql)  (is_lt vs immediate, DVE 4x; col 0 = ones)
so PSUM accumulates, per stream, M[K,L] = joint prefix counts and
M[K,0] = hist(qh); the thermometer makes per-row scans unnecessary.
Rows/cols are interleaved (DG*bin + slot) so each pass's operand AP is a
single uniform-stride free dim.  Birth values accumulate into M0, max
values into M1; og = M0[d0]-M1[d0]-M0[d1]+M1[d1] (all four signs) falls
out of [+sel|-sel] selection matmuls at postproc, then one tiny
triangular matmul gives Cc and a broadcast-add finishes C.

Postprocessing of set s is emitted after compute of set s+1 so the
in-order ACT/DVE/PE streams never stall waiting on PSUM stops.

Sharding: data-parallel over batch, 4 batches per core x 8 cores.
"""

import os
import sys

for _p in ("/opt/trn_rl_repo", os.path.expanduser("~/.axon_site/_ro/trn_rl_repo")):
    if os.path.isdir(_p) and _p not in sys.path:
        sys.path.insert(0, _p)

import numpy as np

import concourse.bass as bass
import concourse.bacc as bacc
import concourse.mybir as mybir
from concourse.tile import TileContext
from concourse.bass_utils import run_bass_kernel_spmd

NCORES = 8
B, C, N = 32, 3, 8192
TT = 256
NG = (B // NCORES) * C        # 12 groups (b,c pairs) per diagram per core
R = float(np.float32(1.0) / np.float32(255.0))
SIZES = [2, 4, 4, 2]          # groups per set (sum = NG, each <= 4);
                              # small first/last sets shrink pipeline
                              # head/tail

F32 = mybir.dt.float32
BF16 = mybir.dt.bfloat16
I16 = mybir.dt.int16
OP = mybir.AluOpType
AF = mybir.ActivationFunctionType
P23 = 8388608.0               # 2^23


def build_nc():
    nc = bacc.Bacc("TRN2", target_bir_lowering=False, debug=False)
    pds = [
        nc.dram_tensor(f"pd{d}", [NG, N, 2], F32, kind="ExternalInput")
        for d in range(2)
    ]
    tri_d = nc.dram_tensor("tri", [16, 16], F32, kind="ExternalInput")
    sel_d = nc.dram_tensor("sel", [128, 1024], F32, kind="ExternalInput")
    out_d = nc.dram_tensor("out", [NG, TT], F32, kind="ExternalOutput")

    with TileContext(nc) as tc:
        with (
            tc.tile_pool(name="consts", bufs=1) as cpool,
            tc.tile_pool(name="src", bufs=2) as spool,
            tc.tile_pool(name="prep", bufs=2) as tpool,
            tc.tile_pool(name="oh", bufs=2) as ohpool,
            tc.tile_pool(name="mm", bufs=2, space="PSUM") as ppool,
            tc.tile_pool(name="pcc", bufs=2, space="PSUM") as ccpool,
            tc.tile_pool(name="post", bufs=2) as qpool,
        ):
            tri = cpool.tile([16, 16], F32)
            sel = cpool.tile([128, 1024], F32)
            # sel col blocks of 256 per DG in (8, 4): [pos | neg] each
            SELOFF = {8: 0, 4: 512}

            goffs = np.cumsum([0] + SIZES[:-1]).tolist()

            def prep(sd):
                """DMA + bin-index computation for set sd."""
                G = SIZES[sd]
                goff = goffs[sd]
                W = 256 * G
                DG = 2 * G

                src = spool.tile([128, 2, G, 128], F32, tag="src", name="src")
                for d in range(2):
                    nc.sync.dma_start(
                        src[:, d, :, :],
                        pds[d]
                        .ap()[goff : goff + G, :, :]
                        .rearrange("g (p x) two -> p g (x two)", p=128),
                    )
                if sd == 0:
                    nc.sync.dma_start(tri[:, :], tri_d.ap())
                    nc.sync.dma_start(sel[:, :], sel_d.ap())

                flat = src[:, :, :, :].rearrange("p d g x -> p (d g x)")
                pairs = src[:, :, :, :].rearrange(
                    "p d g (i two) -> p (d g i) two", two=2
                )
                # deaths <- max(birth, death) in place
                nc.vector.tensor_tensor(
                    pairs[:, :, 1:2], pairs[:, :, 0:1], pairs[:, :, 1:2],
                    OP.max,
                )

                # exact bin index: c = round(v*255) via +2^23 trick; true
                # index is c - [v < t_c] with t_c = fl(c * R)
                tmb = tpool.tile([128, W], F32, tag="tmb", name="tmb")
                cf = tpool.tile([128, W], F32, tag="cf", name="cf")
                tlo = tpool.tile([128, W], F32, tag="tlo", name="tlo")
                ltf = tpool.tile([128, W], F32, tag="ltf", name="ltf")
                q = tpool.tile([128, W], I16, tag="q", name="q")
                nc.scalar.activation(
                    tmb[:, :], flat[:, :], AF.Copy, bias=P23, scale=255.0
                )
                nc.scalar.activation(cf[:, :], tmb[:, :], AF.Copy, bias=-P23)
                nc.scalar.mul(tlo[:, :], cf[:, :], float(R))
                nc.vector.tensor_tensor(
                    ltf[:, :], flat[:, :], tlo[:, :], OP.is_lt
                )
                nc.vector.tensor_tensor(q[:, :], cf[:, :], ltf[:, :], OP.subtract)
                qh = tpool.tile([128, W], I16, tag="qh", name="qh")
                ql = tpool.tile([128, W], I16, tag="ql", name="ql")
                nc.vector.tensor_scalar(
                    qh[:, :], q[:, :], 4, None, OP.logical_shift_right
                )
                nc.vector.tensor_scalar(
                    ql[:, :], q[:, :], 15, None, OP.bitwise_and
                )
                return (sd, G, goff, W, DG, q, qh, ql)

            def bins(st):
                """One-hot/thermometer bin tensors (DVE 4x ops)."""
                sd, G, goff, W, DG, q, qh, ql = st
                # bin tensors, [p, bin, d, g, i, v]; all DVE 4x ops
                A = ohpool.tile([128, 16, 2, G, 64, 2], BF16, tag="A", name="A")
                Bt = ohpool.tile([128, 16, 2, G, 64, 2], BF16, tag="B", name="B")
                Af = A[:, :, :, :, :, :].rearrange("p h d g i v -> p (h d g i v)")
                Bf = Bt[:, :, :, :, :, :].rearrange("p h d g i v -> p (h d g i v)")
                # ones col (L=0): always-true compare runs in DVE 4x mode
                # (memset would be 4x slower and buffer reuse trips the
                # race detector)
                nc.vector.tensor_scalar(
                    Bf[:, 0:W], q[:, :], 30000, None, OP.is_lt
                )
                for h in range(16):
                    nc.vector.tensor_scalar(
                        Af[:, W * h : W * (h + 1)], qh[:, :],
                        h, None, OP.is_equal,
                    )
                for L in range(1, 16):
                    nc.vector.tensor_scalar(
                        Bf[:, W * L : W * (L + 1)], ql[:, :],
                        L, None, OP.is_lt,
                    )
                return (sd, G, goff, DG, A, Bt)

            def mains(bt):
                """Main accumulation matmuls for set sd."""
                sd, G, goff, DG, A, Bt = bt
                # PE: per (v, i) pass contract 128 points; interleaved
                # rows DG*h + slot, cols DG*L + slot (slot = G*d + g)
                M = [ppool.tile([DG * 16, DG * 16], F32, tag=f"M{v}",
                                name=f"M{v}")
                     for v in range(2)]

                def mk_ap(t, i, v):
                    ap = t[:, :, :, :, i, v]
                    # single free dim: addr = 128*(DG*bin + G*d + g)
                    return bass.AP(
                        ap.tensor, ap.offset,
                        [ap.ap[0], [ap.ap[3][0], DG * 16]],
                    )

                for v in range(2):
                    for i in range(64):
                        nc.tensor.matmul(
                            M[v][:, :],
                            mk_ap(A, i, v),
                            mk_ap(Bt, i, v),
                            start=(i == 0),
                            stop=(i == 63),
                        )
                return (sd, G, goff, DG, M)

            def netcp(state):
                """PSUM -> SBUF copies (ACT); one PSUM read per op only."""
                sd, G, goff, DG, M = state
                net0 = qpool.tile([DG * 16, DG * 16], F32, tag="net0",
                                  name="net0")
                net1 = qpool.tile([DG * 16, DG * 16], F32, tag="net1",
                                  name="net1")
                nc.scalar.copy(net0[:, :], M[0][:, :])
                nc.scalar.copy(net1[:, :], M[1][:, :])
                return (sd, G, goff, DG, net0, net1)

            def extract(ns):
                sd, G, goff, DG, net0, net1 = ns
                # og_g = +M0[d0] - M1[d0] - M0[d1] + M1[d1] via selection
                # matmuls; sel[p, 16s+K] = [p == DG*K + s] (negated at +128)
                ogp = ccpool.tile([16, G, 16], F32, tag="ogp", name="ogp")

                def blkcols(net, s):
                    ap = net[:, :]
                    return bass.AP(
                        ap.tensor, ap.offset + s * ap.ap[1][0],
                        [ap.ap[0], [DG * ap.ap[1][0], 16]],
                    )

                for g in range(G):
                    s0, s1 = g, G + g
                    for k, (nt, s, neg) in enumerate(
                        ((net0, s0, 0), (net1, s0, 1),
                         (net0, s1, 1), (net1, s1, 0))
                    ):
                        c0 = SELOFF[DG] + 128 * neg + 16 * s
                        nc.tensor.matmul(
                            ogp[:, g, :],
                            sel[: DG * 16, c0 : c0 + 16],
                            blkcols(nt, s),
                            start=(k == 0), stop=(k == 3),
                        )
                og = qpool.tile([16, G, 16], F32, tag="og", name="og")
                nc.scalar.copy(og[:, :, :], ogp[:, :, :])
                # coarse prefix: Cc[K, g] = sum_{K'<K} hist[K', g]
                ccp = ccpool.tile([16, G], F32, tag="ccp", name="ccp")
                nc.tensor.matmul(
                    ccp[:, :], tri[:, :], og[:, :, 0], start=True, stop=True
                )
                fin = qpool.tile([16, G, 16], F32, tag="fin", name="fin")
                nc.scalar.copy(fin[:, :, 0], ccp[:, :])
                return (sd, G, goff, og, fin)

            def fin_phase(ex):
                sd, G, goff, og, fin = ex
                for g in range(G):
                    nc.vector.tensor_scalar(
                        fin[:, g, 1:16], og[:, g, 1:16],
                        fin[:, g, 0:1], None, OP.add,
                    )
                nc.sync.dma_start(
                    out_d.ap()[goff : goff + G, :].rearrange(
                        "g (K L) -> K g L", K=16
                    ),
                    fin[:, :, :],
                )

            # per-engine interleaving: each engine's in-order queue sees
            # work in data-readiness order (prep(s+1) before netcp(s) on
            # ACT; extract(s) between mains(s) and mains(s+1) on PE; fin(s)
            # after bins(s+1) on DVE)
            NS = len(SIZES)
            mstate = {}
            exstate = {}
            for sd in range(NS):
                st = prep(sd)
                if sd >= 1:
                    exstate[sd - 1] = extract(netcp(mstate[sd - 1]))
                bt = bins(st)
                if sd >= 1:
                    fin_phase(exstate[sd - 1])
                mstate[sd] = mains(bt)
            exstate[NS - 1] = extract(netcp(mstate[NS - 1]))
            fin_phase(exstate[NS - 1])
    nc.compile()
    return nc


_NC = None


def _get_nc():
    global _NC
    if _NC is None:
        _NC = build_nc()
    return _NC


def make_in_maps(pd0, pd1):
    pd0 = np.ascontiguousarray(np.asarray(pd0, dtype=np.float32))
    pd1 = np.ascontiguousarray(np.asarray(pd1, dtype=np.float32))
    tri = (np.arange(16)[:, None] < np.arange(16)[None, :]).astype(np.float32)
    # per DG in (8, 4): sel[DG*K + s, off + 16s + K] = +1, -1 at off+128
    sel = np.zeros((128, 1024), dtype=np.float32)
    for off, dg in ((0, 8), (512, 4)):
        for s in range(dg):
            for K in range(16):
                sel[dg * K + s, off + 16 * s + K] = 1.0
                sel[dg * K + s, off + 128 + 16 * s + K] = -1.0
    bs = B // NCORES
    in_maps = []
    for c in range(NCORES):
        in_maps.append(
            {
                "pd0": np.ascontiguousarray(
                    pd0[bs * c : bs * (c + 1)].reshape(NG, N, 2)
                ),
                "pd1": np.ascontiguousarray(
                    pd1[bs * c : bs * (c + 1)].reshape(NG, N, 2)
                ),
                "tri": tri,
                "sel": sel,
            }
        )
    return in_maps


def kernel(pd0, pd1, trace=False):
    nc = _get_nc()
    in_maps = make_in_maps(pd0, pd1)
    res = run_bass_kernel_spmd(nc, in_maps, list(range(NCORES)), trace=trace)
    bs = B // NCORES
    out = np.concatenate(
        [res.results[c]["out"].reshape(bs, C, TT) for c in range(NCORES)], axis=0
    )
    if trace:
        return out.astype(np.float32), res
    return out.astype(np.float32)
